# revision 30
# baseline (speedup 1.0000x reference)
"""MoE FFN (EnterpriseFFN) Trainium2 kernel -- sparse top-2 dispatch.

8192 tokens x d_model=1024, 8 experts (hidden 512), top-2 gating where every
selected expert is scaled by the SUM of the top-2 softmax gates.

Distribution: data-parallel over tokens -- each of the 8 NeuronCores routes
its 1024 tokens and runs ONLY the selected (token, expert) pairs through the
FFN (~2048 pairs vs 8192 dense), using indirect-DMA row gathers:

  1. Gating on exact fp32 logits (gate_w stationary, x.T streaming), then a
     batched softmax/top-2 on DVE in a [128, 8 chunks x 8 experts] layout.
  2. Routing: sel is PE-transposed to expert-major [8, 1024]; a DVE prefix
     scan ranks each (token, expert) pair; slot = e*CAP + rank. A one-hot
     matrix P[t, j] = (rank[t] == j) turns ranks into a slot->token table:
     idx[e, j] = sum_t (t+1) P[t, j] via 8 PE matmuls per expert, bounced
     through DRAM into [128 part = slot%128, col = slot//128] form. Pad
     slots (idx 0) become an OOB sentinel so their gathers are skipped.
  3. Gather: per 128-slot chunk, one [128, 1]-indexed indirect DMA pulls the
     selected token rows from a host-staged bf16 x copy (the HW primitive:
     one descriptor per partition, offset idx[p], one full row each); PE
     transposes them to feature-major xT (kd-major layout).
  4. Per expert: h = gelu(w1.T @ xT + b1) with 384-wide moving (weight loads
     hidden); layer 2 token-major (lhsT = h chunk, rhs = w2 natural) so
     y[slot, d] streams straight to y_flat (DRAM bf16) with no transpose.
  5. Combine: per-token slot codes (min/max of slot over its selected
     experts) drive two [128, 1] gathers per token chunk;
     out = tokw*(y0+y1) + S.T @ b2, written bf16.

Expert weights are replicated; host pre-casts them (and x) to bf16 and
pre-transposes x for gating -- layout-only work. No collectives; the host
just concatenates the 8 output shards.
"""

import numpy as np
import ml_dtypes

import bass_rust
import concourse.bass as bass
import concourse.tile as tile
from concourse import mybir
from concourse.bass_utils import run_bass_kernel_spmd
from concourse.masks import make_identity
from concourse.tile_rust import add_dep_helper

N_CORES = 8
B, S, D, H, E = 4, 2048, 1024, 512, 8
NTOK = B * S           # 8192 total tokens
TOK = NTOK // N_CORES  # 1024 tokens per core
KD = D // 128          # 8 d_model chunks
KH = H // 128          # 4 hidden chunks
TT = TOK // 128        # 8 token chunks
CAP = 384              # per-expert slot capacity (actual max count is 287);
                       # multiple of 128 so expert slot ranges are whole chunks
CPE = CAP // 128       # slot chunks per expert (3)
NSLOT = E * CAP        # 3072 slots
NCH = NSLOT // 128     # 24 slot chunks
BIGF = 65536.0         # "not selected" sentinel for slot codes
PAD_IDX = 70001
DBG_SKIP_IDXDMA = False        # OOB token id for pad slots (gather skipped)

FP = mybir.dt.float32
BF = mybir.dt.bfloat16
F16 = mybir.dt.float16
I32 = mybir.dt.int32
AF = mybir.ActivationFunctionType
ALU = mybir.AluOpType
AX = mybir.AxisListType


def _legalize_sync_waits(nc, max_waits=1):
    """Split multi-wait instructions for this walrus (1 sync wait per inst)."""
    n_split = 0
    for f in nc.m.functions:
        for bb in f.blocks:
            new_insts = []
            for inst in bb.instructions:
                si = getattr(inst, "sync_info", None)
                if si is not None and len(si.on_wait) > max_waits:
                    waits = list(si.on_wait)
                    for w in waits[max_waits:]:
                        nop = mybir.InstNoOp(
                            name=nc.get_next_instruction_name(), ins=[], outs=[]
                        )
                        nop.engine = inst.engine
                        nop.sync_info = bass_rust.SyncInfo(
                            on_wait=[w], on_update=[]
                        )
                        new_insts.append(nop)
                        n_split += 1
                    inst.sync_info = bass_rust.SyncInfo(
                        on_wait=waits[:max_waits], on_update=list(si.on_update)
                    )
                new_insts.append(inst)
            bb.instructions = new_insts
    return n_split


def _inst(x):
    return getattr(x, "ins", x)


def _emit(tc, xT, xb, gw, w1, b1, w2, b2, idxd, y_flat, out):
    nc = tc.nc

    with (
        tc.tile_pool(name="const", bufs=1) as const_pool,
        tc.tile_pool(name="persist", bufs=1) as persist,
        tc.tile_pool(name="w1pool", bufs=2) as w1pool,
        tc.tile_pool(name="w2pool", bufs=2) as w2pool,
        tc.tile_pool(name="bpool", bufs=3) as bpool,
        tc.tile_pool(name="hpool", bufs=2) as hpool,
        tc.tile_pool(name="ypool", bufs=4) as ypool,
    ):
        ident = const_pool.tile([128, 128], FP, tag="ident")
        make_identity(nc, ident[:])
        ident_b = const_pool.tile([128, 128], BF, tag="ident_b")
        nc.vector.tensor_copy(ident_b[:], ident[:])

        # gate_w [D, E] -> per-d-chunk [128, E] blocks, free-concatenated
        gw_sb = const_pool.tile([128, KD * E], FP, tag="gw")
        for k in range(KD):
            nc.sync.dma_start(
                gw_sb[:, k * E:(k + 1) * E], gw[k * 128:(k + 1) * 128, :]
            )
        # b2 [E, D] natural layout (E on partitions), fp32
        b2sb = const_pool.tile([E, D], FP, tag="b2sb")
        nc.scalar.dma_start(b2sb[:], b2[:, :])

        # NEFF-baked constants (gpsimd.iota is unreliable on HW): slot-code
        # base e*CAP, within-expert slot iota (fp16, replicated rows), and
        # (t+1) token-value columns per token chunk for the positioner
        ecap_d = nc.inline_tensor(
            (np.arange(E, dtype=np.float32) * CAP).reshape(E, 1),
            name="ecap_d",
        )
        ecap_f = const_pool.tile([E, 1], FP, tag="ecap_f")
        nc.scalar.dma_start(ecap_f[:], ecap_d.ap()[:, :])
        jrow_d = nc.inline_tensor(
            np.tile(np.arange(CAP, dtype=np.float16), (128, 1)), name="jrow_d"
        )
        jrow = const_pool.tile([128, CAP], F16, tag="jrow")
        nc.sync.dma_start(jrow[:], jrow_d.ap()[:, :])
        tv = (np.arange(128, dtype=np.float32)[:, None]
              + 128.0 * np.arange(TT, dtype=np.float32)[None, :]) + 1.0
        tvals_d = nc.inline_tensor(tv.astype(np.float16), name="tvals_d")
        tvals = const_pool.tile([128, TT], F16, tag="tvals")
        nc.scalar.dma_start(tvals[:], tvals_d.ap()[:, :])

        # persistent tiles. xT_all is kd-major: [128, kd*NSLOT + slot] bf16
        xT_all = persist.tile([128, KD * NSLOT], BF, tag="xT_all")
        TKW = persist.tile([128, TT], FP, tag="TKW")        # tokw per chunk
        ST = persist.tile([E, TOK], FP, tag="ST")           # sel * tokw
        selT = persist.tile([E, TOK], FP, tag="selT")       # sel 0/1
        slotc = persist.tile([128, 2 * TT], I32, tag="slotc")
        idx_sb = persist.tile([128, NCH], I32, tag="idx_sb")

        engs = [nc.sync, nc.scalar]
        idx_loads = {}

        # ---- stage 1: gating --------------------------------------------
        with (
            tc.tile_pool(name="xg", bufs=1) as xg_pool,
            tc.tile_pool(name="gt", bufs=1) as gt_pool,
            tc.tile_pool(name="gpsum", bufs=1, space="PSUM") as gpsum,
            tc.tile_pool(name="tpsum", bufs=2, space="PSUM") as tpsum,
        ):
            # x.T (host pre-transposed [D, TOK]) -> SBUF, kd-major
            xTg = xg_pool.tile([128, KD * TOK], FP, tag="xTg")
            for kd in range(KD):
                engs[kd % 2].dma_start(
                    xTg[:, kd * TOK:(kd + 1) * TOK],
                    xT[kd * 128:(kd + 1) * 128, :],
                )
            # logits, expert-major: pg[e, t] accumulated over d chunks
            # (two 512-token halves -- a matmul must stay in one PSUM bank)
            pgs = gt_pool.tile([E, TOK], FP, tag="pgs")
            for th in range(2):
                pg = gpsum.tile([E, TOK // 2], FP, tag="pg", name="pg")
                for kd in range(KD):
                    nc.tensor.matmul(
                        pg[:],
                        gw_sb[:, kd * E:(kd + 1) * E],
                        xTg[:, kd * TOK + th * (TOK // 2):
                            kd * TOK + (th + 1) * (TOK // 2)],
                        start=(kd == 0),
                        stop=(kd == KD - 1),
                    )
                nc.vector.tensor_copy(
                    pgs[:, th * (TOK // 2):(th + 1) * (TOK // 2)], pg[:]
                )
            # transpose logits to token-major G [128, t-chunk x e]
            G = gt_pool.tile([128, TT * E], FP, tag="G")
            for t in range(TT):
                pt = tpsum.tile([128, 8], FP, tag="pt", name="ptg")
                nc.tensor.transpose(
                    pt[:], pgs[:, t * 128:(t + 1) * 128], ident[0:E, 0:E]
                )
                nc.vector.tensor_copy(G[:, t * E:(t + 1) * E], pt[:])

            # batched softmax + top-2 on [128, TT, E] views
            g3 = G[:].rearrange("p (t e) -> p t e", e=E)

            def red(out_t, in3, op):
                nc.vector.tensor_reduce(
                    out_t[:].rearrange("p (t e) -> p t e", e=1), in3,
                    axis=AX.X, op=op,
                )

            def bc(t_):  # [128, TT] -> broadcast [128, TT, E]
                return t_[:].rearrange("p (t e) -> p t e", e=1).to_broadcast(
                    [128, TT, E]
                )

            M = gt_pool.tile([128, TT], FP, tag="M")
            red(M, g3, ALU.max)
            Dm = gt_pool.tile([128, TT * E], FP, tag="Dm")
            d3 = Dm[:].rearrange("p (t e) -> p t e", e=E)
            nc.vector.tensor_tensor(d3, g3, bc(M), op=ALU.subtract)
            Ex = gt_pool.tile([128, TT * E], FP, tag="Ex")
            nc.scalar.activation(Ex[:], Dm[:], AF.Exp)
            e3 = Ex[:].rearrange("p (t e) -> p t e", e=E)
            SS = gt_pool.tile([128, TT], FP, tag="SS")
            red(SS, e3, ALU.add)
            R = gt_pool.tile([128, TT], FP, tag="R")
            nc.vector.reciprocal(R[:], SS[:])
            Gm = gt_pool.tile([128, TT * E], FP, tag="Gm")
            gm3 = Gm[:].rearrange("p (t e) -> p t e", e=E)
            nc.vector.tensor_tensor(gm3, e3, bc(R), op=ALU.mult)
            M1 = gt_pool.tile([128, TT], FP, tag="M1")
            red(M1, gm3, ALU.max)
            IS1 = gt_pool.tile([128, TT * E], FP, tag="IS1")
            is13 = IS1[:].rearrange("p (t e) -> p t e", e=E)
            nc.vector.tensor_tensor(is13, gm3, bc(M1), op=ALU.is_ge)
            G2 = gt_pool.tile([128, TT * E], FP, tag="G2")
            nc.vector.tensor_scalar(G2[:], IS1[:], -2.0, None, op0=ALU.mult)
            nc.vector.tensor_tensor(G2[:], G2[:], Gm[:], op=ALU.add)
            M2 = gt_pool.tile([128, TT], FP, tag="M2")
            red(M2, G2[:].rearrange("p (t e) -> p t e", e=E), ALU.max)
            nc.vector.tensor_tensor(TKW[:], M1[:], M2[:], op=ALU.add)
            SEL = gt_pool.tile([128, TT * E], FP, tag="SEL")
            sel3 = SEL[:].rearrange("p (t e) -> p t e", e=E)
            nc.vector.tensor_tensor(sel3, gm3, bc(M2), op=ALU.is_ge)
            SW = gt_pool.tile([128, TT * E], FP, tag="SW")
            nc.vector.tensor_tensor(
                SW[:].rearrange("p (t e) -> p t e", e=E), sel3, bc(TKW),
                op=ALU.mult,
            )

            # transpose sel / sel*tokw to expert-major [E, TOK]
            for t in range(TT):
                ts = slice(t * 128, (t + 1) * 128)
                p1 = tpsum.tile([128, 128], FP, tag="pt", name="p1")
                nc.tensor.transpose(
                    p1[0:E, :], SEL[:, t * E:(t + 1) * E], ident[:]
                )
                nc.vector.tensor_copy(selT[:, ts], p1[0:E, :])
                p2 = tpsum.tile([128, 128], FP, tag="pt", name="p2")
                nc.tensor.transpose(
                    p2[0:E, :], SW[:, t * E:(t + 1) * E], ident[:]
                )
                nc.vector.tensor_copy(ST[:, ts], p2[0:E, :])

            # ---- stage 2: routing ---------------------------------------
            with (
                tc.tile_pool(name="rt", bufs=1) as rt_pool,
                tc.tile_pool(name="pp16", bufs=2) as pp16_pool,
                tc.tile_pool(name="irow", bufs=2) as irow_pool,
                tc.tile_pool(name="ipsum", bufs=2, space="PSUM") as ipsum,
            ):
                pos = rt_pool.tile([E, TOK], FP, tag="pos")
                nc.vector.tensor_tensor_scan(
                    pos[:], selT[:], selT[:], 0.0, op0=ALU.add, op1=ALU.bypass
                )
                # exclusive rank
                nc.vector.tensor_tensor(pos[:], pos[:], selT[:],
                                        op=ALU.subtract)
                ok = rt_pool.tile([E, TOK], FP, tag="ok")
                nc.vector.tensor_scalar(ok[:], pos[:], float(CAP), None,
                                        op0=ALU.is_lt)
                nc.vector.tensor_tensor(ok[:], ok[:], selT[:], op=ALU.mult)
                code = rt_pool.tile([E, TOK], FP, tag="code")
                nc.vector.tensor_scalar(code[:], pos[:], ecap_f[:, 0:1], None,
                                        op0=ALU.add)
                # per-token slot codes: cmin = ok ? code : BIG,
                # cmax = ok ? code : -1
                cmin = rt_pool.tile([E, TOK], FP, tag="cmin")
                nc.vector.tensor_scalar(cmin[:], code[:], -BIGF, None,
                                        op0=ALU.add)
                nc.vector.tensor_tensor(cmin[:], cmin[:], ok[:], op=ALU.mult)
                nc.vector.tensor_scalar(cmin[:], cmin[:], BIGF, None,
                                        op0=ALU.add)
                cmax = rt_pool.tile([E, TOK], FP, tag="cmax")
                nc.vector.tensor_scalar(cmax[:], code[:], 1.0, None,
                                        op0=ALU.add)
                nc.vector.tensor_tensor(cmax[:], cmax[:], ok[:], op=ALU.mult)
                nc.vector.tensor_scalar(cmax[:], cmax[:], -1.0, None,
                                        op0=ALU.add)
                # posm = ok ? rank : -1, transposed token-major
                posm = rt_pool.tile([E, TOK], FP, tag="posm")
                nc.vector.tensor_scalar(posm[:], pos[:], 1.0, None,
                                        op0=ALU.add)
                nc.vector.tensor_tensor(posm[:], posm[:], ok[:], op=ALU.mult)
                nc.vector.tensor_scalar(posm[:], posm[:], -1.0, None,
                                        op0=ALU.add)
                posm_tok = rt_pool.tile([128, TT * E], FP, tag="posm_tok")
                for t in range(TT):
                    pp = tpsum.tile([128, 8], FP, tag="pt", name="pp")
                    nc.tensor.transpose(
                        pp[:], posm[:, t * 128:(t + 1) * 128],
                        ident[0:E, 0:E],
                    )
                    nc.vector.tensor_copy(
                        posm_tok[:, t * E:(t + 1) * E], pp[:]
                    )

                for t in range(TT):
                    ts = slice(t * 128, (t + 1) * 128)
                    pc0 = tpsum.tile([128, 8], FP, tag="pt", name="pc0")
                    nc.tensor.transpose(pc0[:], cmin[:, ts], ident[0:E, 0:E])
                    pc1 = tpsum.tile([128, 8], FP, tag="pt", name="pc1")
                    nc.tensor.transpose(pc1[:], cmax[:, ts], ident[0:E, 0:E])
                    s0 = rt_pool.tile([128, 1], FP, tag="s0", name="s0")
                    nc.vector.tensor_reduce(s0[:], pc0[:], axis=AX.X,
                                            op=ALU.min)
                    s1 = rt_pool.tile([128, 1], FP, tag="s1", name="s1")
                    nc.vector.tensor_reduce(s1[:], pc1[:], axis=AX.X,
                                            op=ALU.max)
                    nc.vector.tensor_copy(slotc[:, 2 * t:2 * t + 1], s0[:])
                    nc.vector.tensor_copy(slotc[:, 2 * t + 1:2 * t + 2], s1[:])

                # positioner: idx row [1, CAP] per expert = sum_t (t+1) *
                # P[t, j], bounced via DRAM into [128, CPE] gather tables
                for e in range(E):
                    irow_ps = ipsum.tile([1, CAP], FP, tag="irp",
                                         name="irow_ps")
                    for t in range(TT):
                        Pt = pp16_pool.tile([128, CAP], F16, tag="Pt",
                                            name="Pt")
                        nc.vector.tensor_scalar(
                            Pt[:], jrow[:],
                            posm_tok[:, t * E + e:t * E + e + 1],
                            None, op0=ALU.is_equal,
                        )
                        nc.tensor.matmul(
                            irow_ps[:],
                            tvals[:, t:t + 1],
                            Pt[:],
                            start=(t == 0),
                            stop=(t == TT - 1),
                        )
                    irow_i = irow_pool.tile([1, CAP], I32, tag="irow_i",
                                            name="irow_i")
                    nc.vector.tensor_copy(irow_i[:], irow_ps[:])
                    if DBG_SKIP_IDXDMA:
                        nc.vector.memset(idx_sb[:, CPE * e:CPE * (e + 1)], 0)
                    else:
                        wr = engs[e % 2].dma_start(idxd[e:e + 1, :], irow_i[:, :])
                        ldi = engs[(e + 1) % 2].dma_start(
                            idx_sb[:, CPE * e:CPE * (e + 1)],
                            idxd[e].rearrange("(c p) -> p c", p=128),
                        )
                        add_dep_helper(_inst(ldi), _inst(wr),
                                       reason="idx row bounce")
                        idx_loads[e] = ldi
                # pads carry 0 (no (t+1) hit); remap idx = raw-1, pad ->
                # PAD_IDX (> TOK, gather-skipped)
                im = rt_pool.tile([128, NCH], I32, tag="im")
                nc.vector.tensor_scalar(im[:], idx_sb[:], 0, None,
                                        op0=ALU.is_equal)
                nc.vector.tensor_scalar(im[:], im[:], PAD_IDX, None,
                                        op0=ALU.mult)
                nc.vector.tensor_scalar(idx_sb[:], idx_sb[:], -1, None,
                                        op0=ALU.add)
                nc.vector.tensor_tensor(idx_sb[:], idx_sb[:], im[:],
                                        op=ALU.add)

        # ---- stage 3+4: gather, transpose, per-expert FFN ----------------
        loaded = {}

        def _load_w(e):
            w1t = w1pool.tile([128, KD * H], BF, tag="w1t", name="w1t")
            for k in range(KD):
                engs[k % 2].dma_start(
                    w1t[:, k * H:(k + 1) * H],
                    w1[e, k * 128:(k + 1) * 128, :],
                )
            w2t = w2pool.tile([128, KH * D], BF, tag="w2t", name="w2t")
            for k in range(KH):
                engs[k % 2].dma_start(
                    w2t[:, k * D:(k + 1) * D],
                    w2[e, k * 128:(k + 1) * 128, :],
                )
            b1t = bpool.tile([128, KH], FP, tag="b1t", name="b1t")
            nc.gpsimd.dma_start(b1t[:], b1[e].rearrange("(k p) -> p k", p=128))
            loaded[e] = (w1t, w2t, b1t)

        _load_w(0)
        _load_w(1)

        y_writes = []
        with (
            tc.tile_pool(name="xgt", bufs=3) as xgt_pool,
            tc.tile_pool(name="xpsum", bufs=2, space="PSUM") as xpsum,
            tc.tile_pool(name="hpsum", bufs=3, space="PSUM") as hpsum,
            tc.tile_pool(name="ypsum", bufs=3, space="PSUM") as ypsum,
        ):
            for e in range(E):
                for c in range(CPE):
                    gc = e * CPE + c
                    xgt = xgt_pool.tile([128, D], BF, tag="xgt", name="xgt")
                    nc.gpsimd.indirect_dma_start(
                        out=xgt[:],
                        out_offset=None,
                        in_=xb[:, :],
                        in_offset=bass.IndirectOffsetOnAxis(
                            ap=idx_sb[:, gc:gc + 1], axis=0
                        ),
                        bounds_check=TOK - 1,
                        oob_is_err=False,
                    )
                    # PE-transpose gathered rows [slot, d] -> feature-major
                    for kd in range(KD):
                        px = xpsum.tile([128, 128], BF, tag="px", name="px")
                        nc.tensor.transpose(
                            px[:], xgt[:, kd * 128:(kd + 1) * 128],
                            ident_b[:],
                        )
                        dsl = xT_all[:, kd * NSLOT + gc * 128:
                                     kd * NSLOT + (gc + 1) * 128]
                        if kd % 2 == 0:
                            nc.scalar.copy(dsl, px[:])
                        else:
                            nc.vector.tensor_copy(dsl, px[:])
                w1t, w2t, b1t = loaded.pop(e)
                if e + 2 < E:
                    _load_w(e + 2)
                so = e * CAP
                ht = hpool.tile([128, KH * CAP], BF, tag="ht", name="ht")
                for mh in range(KH):
                    ph = hpsum.tile([128, CAP], FP, tag="ph", name="ph")
                    for kd in range(KD):
                        nc.tensor.matmul(
                            ph[:],
                            w1t[:, kd * H + mh * 128:kd * H + (mh + 1) * 128],
                            xT_all[:, kd * NSLOT + so:kd * NSLOT + so + CAP],
                            start=(kd == 0),
                            stop=(kd == KD - 1),
                        )
                    nc.scalar.activation(
                        ht[:, mh * CAP:(mh + 1) * CAP], ph[:], AF.Gelu,
                        bias=b1t[:, mh:mh + 1],
                    )
                # layer 2, token-major: y[slot, d] = h.T @ w2
                for c in range(CPE):
                    yt = ypool.tile([128, D], BF, tag="yt", name="yt")
                    for hf in range(2):
                        py = ypsum.tile([128, D // 2], FP, tag="py",
                                        name="py")
                        for kh in range(KH):
                            nc.tensor.matmul(
                                py[:],
                                ht[:, kh * CAP + c * 128:
                                   kh * CAP + (c + 1) * 128],
                                w2t[:, kh * D + hf * (D // 2):
                                    kh * D + (hf + 1) * (D // 2)],
                                start=(kh == 0),
                                stop=(kh == KH - 1),
                            )
                        nc.vector.tensor_copy(
                            yt[:, hf * (D // 2):(hf + 1) * (D // 2)], py[:],
                        )
                    yw = engs[(e + c) % 2].dma_start(
                        y_flat[so + c * 128:so + (c + 1) * 128, :], yt[:]
                    )
                    y_writes.append(yw)

        # ---- stage 5: combine -------------------------------------------
        with (
            tc.tile_pool(name="cg", bufs=3) as cg_pool,
            tc.tile_pool(name="co", bufs=3) as co_pool,
            tc.tile_pool(name="fence", bufs=1) as fence_pool,
            tc.tile_pool(name="bpsum", bufs=4, space="PSUM") as bpsum,
        ):
            # fence: combine gathers read y_flat rows written by the 24 y
            # DMAs on other queues -- funnel those deps through one no-op
            fsc = fence_pool.tile([1, 1], FP, tag="fsc")
            fence = nc.vector.memset(fsc[:], 0.0)
            for yw in y_writes:
                add_dep_helper(_inst(fence), _inst(yw), reason="y->fence")
            for t in range(TT):
                ts = slice(t * 128, (t + 1) * 128)
                g01 = cg_pool.tile([128, 2 * D], BF, tag="g01", name="g01")
                for k in range(2):
                    cgi = nc.gpsimd.indirect_dma_start(
                        out=g01[:, k * D:(k + 1) * D],
                        out_offset=None,
                        in_=y_flat[:, :],
                        in_offset=bass.IndirectOffsetOnAxis(
                            ap=slotc[:, 2 * t + k:2 * t + k + 1], axis=0
                        ),
                    )
                    add_dep_helper(_inst(cgi), _inst(fence),
                                   reason="fence->combine")
                acc = co_pool.tile([128, D], FP, tag="acc", name="acc")
                nc.vector.tensor_tensor(
                    acc[:], g01[:, 0:D], g01[:, D:2 * D], op=ALU.add,
                )
                nc.vector.tensor_scalar(
                    acc[:], acc[:], TKW[:, t:t + 1], None, op0=ALU.mult
                )
                accb = co_pool.tile([128, D], BF, tag="accb", name="accb")
                for hf in range(2):
                    pb = bpsum.tile([128, D // 2], FP, tag="pb", name="pb")
                    nc.tensor.matmul(
                        pb[:],
                        ST[:, ts],
                        b2sb[:, hf * (D // 2):(hf + 1) * (D // 2)],
                        start=True,
                        stop=True,
                    )
                    nc.vector.tensor_tensor(
                        accb[:, hf * (D // 2):(hf + 1) * (D // 2)],
                        acc[:, hf * (D // 2):(hf + 1) * (D // 2)],
                        pb[:],
                        op=ALU.add,
                    )
                engs[t % 2].dma_start(out[ts, :], accb[:])


_CACHED_NC = None


def _build(legalize=True):
    global _CACHED_NC
    if _CACHED_NC is not None and legalize:
        return _CACHED_NC
    nc = bass.Bass(
        "TRN2", target_bir_lowering=False, debug=False, num_devices=N_CORES
    )
    xT = nc.dram_tensor("xT", [D, TOK], FP, kind="ExternalInput").ap()
    xb = nc.dram_tensor("xb", [TOK, D], BF, kind="ExternalInput").ap()
    gw = nc.dram_tensor("gate_w", [D, E], FP, kind="ExternalInput").ap()
    w1 = nc.dram_tensor("w1", [E, D, H], BF, kind="ExternalInput").ap()
    b1 = nc.dram_tensor("b1", [E, H], FP, kind="ExternalInput").ap()
    w2 = nc.dram_tensor("w2", [E, H, D], BF, kind="ExternalInput").ap()
    b2 = nc.dram_tensor("b2", [E, D], FP, kind="ExternalInput").ap()
    idxd = nc.dram_tensor("idxd", [E, CAP], I32, kind="Internal").ap()
    y_flat = nc.dram_tensor("y_flat", [NSLOT, D], BF, kind="Internal").ap()
    out = nc.dram_tensor("out", [TOK, D], BF, kind="ExternalOutput").ap()
    with tile.TileContext(nc) as tc:
        _emit(tc, xT, xb, gw, w1, b1, w2, b2, idxd, y_flat, out)
    if not legalize:
        return nc
    _legalize_sync_waits(nc)
    _CACHED_NC = nc
    return nc


def run(inputs, **spmd_kwargs):
    """Shard, run on 8 cores, unshard. Returns (out [B,S,D], results)."""
    nc = _build()
    xf = np.ascontiguousarray(
        np.asarray(inputs["x"], dtype=np.float32).reshape(NTOK, D)
    )
    shared = {
        "gate_w": np.ascontiguousarray(
            np.asarray(inputs["gate_w"], dtype=np.float32)
        ),
        "w1": np.ascontiguousarray(
            np.asarray(inputs["w1"], dtype=np.float32)
        ).astype(ml_dtypes.bfloat16),
        "b1": np.ascontiguousarray(np.asarray(inputs["b1"], dtype=np.float32)),
        "w2": np.ascontiguousarray(
            np.asarray(inputs["w2"], dtype=np.float32)
        ).astype(ml_dtypes.bfloat16),
        "b2": np.ascontiguousarray(np.asarray(inputs["b2"], dtype=np.float32)),
    }
    in_maps = []
    for c in range(N_CORES):
        xs = xf[c * TOK:(c + 1) * TOK]  # [TOK, D]
        in_maps.append({
            "xT": np.ascontiguousarray(xs.T),
            "xb": np.ascontiguousarray(xs).astype(ml_dtypes.bfloat16),
            **shared,
        })
    res = run_bass_kernel_spmd(nc, in_maps, list(range(N_CORES)), **spmd_kwargs)
    out = np.concatenate(
        [res.results[c]["out"].astype(np.float32) for c in range(N_CORES)],
        axis=0,
    )
    return out.reshape(B, S, D), res


def kernel(**inputs):
    out, _ = run(inputs)
    return out
